# revision 14
# baseline (speedup 1.0000x reference)
"""Trainium2 Bass kernel for HadamardPackedLinear.

Math (reference):
    y[t, 128*h + o] = beta[o]/32 * sum_g Hpm[g,h] * sum_i xm[t,g,i] * w[g,o,i]
    xm[t,g,i] = sum_g' x[t, 128g'+i] * Hpm[g',g]
with Hpm the +-1 Sylvester Hadamard (H = Hpm/sqrt(32), applied twice -> 1/32).

Three PE stages per 256-token chunk (4 chunks/core), everything fp16 on the
PE with fp32 PSUM accumulate:

  S1: lhsT = I4 (x) Hpm  [ (d,g) x (d,h) ],  rhs = x [ (d,g), (u,i_hi,v) ]
      (host pre-shuffles x; tokens t = 2u+v so fp16 pairs over v are
      memory-adjacent)  ->  psum1[(d,h), (u,i_hi,v)]
  drain1 (ACT): psum1 -> xm1 fp16, contiguous multibank copies
  T1 (DVE, fp32-bitcast pair transpose): xm1 -> xm2[i, (u,h,v)]
      (32x32 block transpose in fp32 units moves 2 fp16/lane-cycle)
  S2: per g: lhsT = w2'[g] = w[g].T * beta/32  [i, o],
      rhs = xm2 cols {64u + 2g + v}  (2-elem runs, stride 64)
      -> psum2[o, t] per g, 8 g per 4-bank psum tile
  gather2 (ACT): multibank (t-outer, g-inner) gather -> yp[o, (t,h)] fp16
      (8-elem-run writes, cross-bank strided reads run at full ACT rate)
  T2 (DVE fp16 32x32 transpose): yp -> ypt[(d3,h), (t,o_hi)]
  S3: lhsT = I4 (x) Hpm again, rhs = ypt contiguous
      -> psum3[(d3',h'), (t,o_hi)]
  drain3 (ACT/DVE alternating): psum3 -> y_sb fp16 -> DMA out

Host: x cast to fp16 + column shuffle; output unshuffle + fp32 cast.
Sharding: data-parallel over tokens, 8 cores x 1024 tokens. No collectives.
"""

import sys

for _p in ("/opt/trn_rl_repo", "/root/.axon_site/_ro/trn_rl_repo"):
    if _p not in sys.path:
        sys.path.append(_p)

import numpy as np

import concourse.bass as bass  # noqa: E402
import concourse.mybir as mybir  # noqa: E402
import concourse.tile as tile  # noqa: E402
from concourse import bacc  # noqa: E402
from concourse.bass_utils import run_bass_kernel_spmd  # noqa: E402

F32 = mybir.dt.float32
F16 = mybir.dt.float16

N_CORES = 8
B, T, D = 4, 2048, 4096
A = 32              # algebra dim (hadamard size)
IN_O = 128
OUT_O = 128
TOK = (B * T) // N_CORES    # tokens per core = 1024
TC = 256                    # tokens per chunk
NCH = TOK // TC             # 4 chunks
CCOLS = TC * 32             # 8192 sbuf cols per chunk

_CACHE = {}


def _build_program():
    nc = bacc.Bacc(None, target_bir_lowering=False)

    x_d = nc.dram_tensor("x_shuf", [128, TOK * 32], F16, kind="ExternalInput")
    h1_d = nc.dram_tensor("h1m", [128, 128], F16, kind="ExternalInput")
    w2_d = nc.dram_tensor("w2m", [128, A * OUT_O], F16, kind="ExternalInput")
    y_d = nc.dram_tensor("y", [128, TOK * 32], F16, kind="ExternalOutput")

    with tile.TileContext(nc) as tc:
        with (
            tc.tile_pool(name="const", bufs=1) as constp,
            tc.tile_pool(name="xin", bufs=2) as xinp,
            tc.tile_pool(name="xm1", bufs=2) as xm1p,
            tc.tile_pool(name="xm2", bufs=2) as xm2p,
            tc.tile_pool(name="yp", bufs=2) as ypp,
            tc.tile_pool(name="ypt", bufs=2) as yptp,
            tc.tile_pool(name="ysb", bufs=2) as ysbp,
            tc.tile_pool(name="psA", bufs=3, space="PSUM") as psAp,
            tc.tile_pool(name="ps2", bufs=1, space="PSUM") as ps2p,
        ):
            h1_t = constp.tile([128, 128], F16)
            nc.sync.dma_start(out=h1_t[:], in_=h1_d[:])
            w2_t = constp.tile([128, A * OUT_O], F16)

            def load_x(c):
                subs = []
                for s in range(4):
                    xs_t = xinp.tile([128, 2048], F16, name="x_sub")
                    nc.sync.dma_start(
                        out=xs_t[:],
                        in_=x_d[:, c * CCOLS + s * 2048 : c * CCOLS + (s + 1) * 2048],
                    )
                    subs.append(xs_t)
                return subs

            def s1_round(ctx, r, pool=None):
                x_subs, xm1_32, xm2_32 = ctx
                ps = (pool.tile([128, 1024], F32, name="ps2t") if pool is not None
                      else psAp.tile([128, 1024], F32, name="psq"))
                for m in range(2):
                    off = (r % 2) * 1024 + m * 512
                    nc.tensor.matmul(
                        ps[:, m * 512 : m * 512 + 512],
                        h1_t[:],
                        x_subs[r // 2][:, off : off + 512],
                        start=True,
                        stop=True,
                    )
                nc.scalar.copy(
                    xm1_32[:, r * 512 : (r + 1) * 512].bitcast(F16), ps[:]
                )
                if r % 4 == 3:
                    s = r // 4
                    # xm2_32[i, w*64 + h*2 + s] = T(xm1 half s)
                    dstv = xm2_32.rearrange("p (w e q) -> p w q e", e=32, q=2)
                    nc.vector.transpose(
                        dstv[:, :, s : s + 1, :],
                        xm1_32[:, s * 2048 : (s + 1) * 2048],
                    )

            def s2_round(ctx, r):
                xm2_v, ypv = ctx
                ps2 = ps2p.tile([128, 1024], F32, name="ps2t")
                for j in range(4):
                    g = r * 4 + j
                    nc.tensor.matmul(
                        ps2[:, j * 256 : (j + 1) * 256],
                        w2_t[:, g * 128 : (g + 1) * 128],
                        xm2_v[:, :, 4 * g : 4 * g + 4],
                        start=True,
                        stop=True,
                    )
                # gather: yp[o, 32t + 4r + j] = ps2[o, 256j + t]
                geng = nc.vector.tensor_copy if r in (2, 5) else nc.scalar.copy
                geng(
                    ypv[:, :, 4 * r : 4 * r + 4],
                    ps2[:].rearrange("p (j t) -> p t j", j=4),
                )

            def s3_round(ctx, r, c, pool=None):
                ypt_t, y_sb = ctx
                ps = (pool.tile([128, 1024], F32, name="ps2t") if pool is not None
                      else psAp.tile([128, 1024], F32, name="psq"))
                for m in range(2):
                    off = r * 1024 + m * 512
                    nc.tensor.matmul(
                        ps[:, m * 512 : m * 512 + 512],
                        h1_t[:],
                        ypt_t[:, off : off + 512],
                        start=True,
                        stop=True,
                    )
                dst = y_sb[:, r * 1024 : (r + 1) * 1024]
                if r in (1, 4, 7):
                    nc.scalar.copy(dst, ps[:])
                else:
                    nc.vector.tensor_copy(dst, ps[:])
                if r % 2 == 1:
                    nc.sync.dma_start(
                        out=y_d[:, c * CCOLS + (r - 1) * 1024 : c * CCOLS + (r + 1) * 1024],
                        in_=y_sb[:, (r - 1) * 1024 : (r + 1) * 1024],
                    )

            # software-pipelined, round-interleaved: S1(c) | S2(c-1) | S3(c-2)
            xs = {0: load_x(0)}
            nc.sync.dma_start(out=w2_t[:], in_=w2_d[:])
            c1 = {}   # chunk -> stage1 ctx (x, xm1_32, xm2_32)
            c2 = {}   # chunk -> (xm2_t, yp_t, ypt_t)
            c3 = {}   # chunk -> (ypt_t, y_sb)
            for it in range(NCH + 2):
                if it < NCH:
                    xm1_t = xm1p.tile([128, CCOLS], F16, name="xm1_t")
                    xm2_t = xm2p.tile([128, CCOLS], F16, name="xm2_t")
                    c1[it] = (xs.pop(it), xm1_t[:].bitcast(F32), xm2_t[:].bitcast(F32))
                    c2[it] = [xm2_t, None, None]
                if 0 <= it - 1 < NCH:
                    yp_t = ypp.tile([128, CCOLS], F16, name="yp_t")
                    ypt_t = yptp.tile([128, CCOLS], F16, name="ypt_t")
                    c2[it - 1][1] = yp_t
                    c2[it - 1][2] = ypt_t
                if 0 <= it - 2 < NCH:
                    y_sb = ysbp.tile([128, CCOLS], F16, name="y_sb")
                    c3[it - 2] = (c2[it - 2][2], y_sb)

                s2ctx = None
                if 0 <= it - 1 < NCH:
                    xm2_t, yp_t, _ = c2[it - 1]
                    s2ctx = (
                        xm2_t[:].rearrange("p (w q) -> p w q", q=128),
                        yp_t[:].rearrange("p (t h) -> p t h", h=32),
                    )
                s2_active = s2ctx is not None
                for r in range(8):
                    if it < NCH:
                        p1 = None if s2_active else (ps2p if r % 2 else None)
                        s1_round(c1[it], r, p1)
                    if s2_active:
                        s2_round(s2ctx, r)
                    if 0 <= it - 2 < NCH:
                        p3 = None if s2_active else (ps2p if r % 2 == 0 else None)
                        s3_round(c3[it - 2], r, it - 2, p3)
                    if r == 3 and it + 1 < NCH:
                        xs[it + 1] = load_x(it + 1)

                # T2 for chunk it-1 (needs all gathers of that chunk)
                if 0 <= it - 1 < NCH:
                    _, yp_t, ypt_t = c2[it - 1]
                    for s in range(2):
                        nc.vector.transpose(
                            ypt_t[:, s * 4096 : (s + 1) * 4096],
                            yp_t[:, s * 4096 : (s + 1) * 4096],
                        )
                if 0 <= it - 2 < NCH:
                    c3.pop(it - 2)
                    c2.pop(it - 2)

    nc.compile()
    return nc


def _host_prep(x, weight_packed, beta, H):
    x = np.asarray(x, dtype=np.float32)
    weight_packed = np.asarray(weight_packed, dtype=np.uint8)
    beta = np.asarray(beta, dtype=np.float32)
    H = np.asarray(H, dtype=np.float32)

    hpm = np.where(H > 0, 1.0, -1.0).astype(np.float32)

    # unpack ternary weights exactly like the reference
    p = weight_packed
    v0 = ((p >> 6) & 3).astype(np.int8) - 1
    v1 = ((p >> 4) & 3).astype(np.int8) - 1
    v2 = ((p >> 2) & 3).astype(np.int8) - 1
    v3 = (p & 3).astype(np.int8) - 1
    w = np.stack([v0, v1, v2, v3], axis=-1).reshape(A, OUT_O, IN_O).astype(np.float32)

    # stage 1/3 stationary: h1m[32d+g, 32d'+h] = delta_dd' * hpm[g,h]
    h1m = np.zeros((4, A, 4, A), dtype=np.float32)
    for d in range(4):
        h1m[d, :, d, :] = hpm
    h1m = h1m.reshape(128, 128).astype(np.float16)

    # stage 2 stationary: w2m[i, 128g + o] = w[g,o,i] * beta[o] / 32
    w2 = w * (beta[None, :, None] / 32.0)
    w2m = np.ascontiguousarray(w2.transpose(2, 0, 1).reshape(IN_O, A * OUT_O)).astype(
        np.float16
    )

    # per-core pre-shuffled x (fp16):
    # x_shuf[32d+g, 8192c + 64u + 2*i_hi + v] = x[t0 + 256c + 2u + v, 128g+32d+i_hi]
    xf = x.reshape(B * T, D).astype(np.float16)
    x_shards = []
    for core in range(N_CORES):
        xc = xf[core * TOK : (core + 1) * TOK]          # [TOK, 4096]
        # t = 256c + 4w + 2*half + va ; col = 8192c + 4096*half + 64w + 2*i_hi + va
        xc = xc.reshape(NCH, TC // 4, 2, 2, A, 4, 32)   # [c, w, half, va, g, d, i_hi]
        xc = xc.transpose(5, 4, 0, 2, 1, 6, 3)          # [d, g, c, half, w, i_hi, va]
        x_shards.append(np.ascontiguousarray(xc.reshape(128, TOK * 32)))

    return x_shards, h1m, w2m


def kernel(x, weight_packed, beta, H):
    x_shards, h1m, w2m = _host_prep(x, weight_packed, beta, H)

    if "nc" not in _CACHE:
        _CACHE["nc"] = _build_program()
    nc = _CACHE["nc"]

    in_maps = [
        {"x_shuf": x_shards[c], "h1m": h1m, "w2m": w2m} for c in range(N_CORES)
    ]
    res = run_bass_kernel_spmd(nc, in_maps, core_ids=list(range(N_CORES)))

    # y_d[32d3'+h', 8192c + 32t + o_hi] = y[t0+256c+t, 128h' + 32d3' + o_hi]
    out = np.empty((B * T, D), dtype=np.float32)
    for core in range(N_CORES):
        yd = np.asarray(res.results[core]["y"])          # [128, TOK*32] fp16
        arr = yd.reshape(4, A, NCH, TC, 32)              # [d3', h', c, t, o_hi]
        arr = arr.transpose(2, 3, 1, 0, 4).reshape(TOK, D)  # [c,t][h',d3',o_hi]
        out[core * TOK : (core + 1) * TOK] = arr.astype(np.float32)
    return out.reshape(B, T, D)


# revision 15
# speedup vs baseline: 1.1561x; 1.1561x over previous
"""Trainium2 Bass kernel for HadamardPackedLinear.

Math (reference):
    y[t, 128*h + o] = beta[o]/32 * sum_g Hpm[g,h] * sum_i xm[t,g,i] * w[g,o,i]
    xm[t,g,i] = sum_g' x[t, 128g'+i] * Hpm[g',g]
with Hpm the +-1 Sylvester Hadamard (H = Hpm/sqrt(32), applied twice -> 1/32).

Three PE stages per 256-token chunk (4 chunks/core), everything fp16 on the
PE with fp32 PSUM accumulate:

  S1: lhsT = I4 (x) Hpm  [ (d,g) x (d,h) ],  rhs = x [ (d,g), (u,i_hi,v) ]
      (host pre-shuffles x; tokens t = 2u+v so fp16 pairs over v are
      memory-adjacent)  ->  psum1[(d,h), (u,i_hi,v)]
  drain1 (ACT): psum1 -> xm1 fp16, contiguous multibank copies
  T1 (DVE, fp32-bitcast pair transpose): xm1 -> xm2[i, (u,h,v)]
      (32x32 block transpose in fp32 units moves 2 fp16/lane-cycle)
  S2: per g: lhsT = w2'[g] = w[g].T * beta/32  [i, o],
      rhs = xm2 cols {64u + 2g + v}  (2-elem runs, stride 64)
      -> psum2[o, t] per g, 8 g per 4-bank psum tile
  gather2 (ACT): multibank (t-outer, g-inner) gather -> yp[o, (t,h)] fp16
      (8-elem-run writes, cross-bank strided reads run at full ACT rate)
  T2 (DVE fp16 32x32 transpose): yp -> ypt[(d3,h), (t,o_hi)]
  S3: lhsT = I4 (x) Hpm again, rhs = ypt contiguous
      -> psum3[(d3',h'), (t,o_hi)]
  drain3 (ACT/DVE alternating): psum3 -> y_sb fp16 -> DMA out

Host: x cast to fp16 + column shuffle; output unshuffle + fp32 cast.
Sharding: data-parallel over tokens, 8 cores x 1024 tokens. No collectives.
"""

import sys

for _p in ("/opt/trn_rl_repo", "/root/.axon_site/_ro/trn_rl_repo"):
    if _p not in sys.path:
        sys.path.append(_p)

import numpy as np

import concourse.bass as bass  # noqa: E402
import concourse.mybir as mybir  # noqa: E402
import concourse.tile as tile  # noqa: E402
from concourse import bacc  # noqa: E402
from concourse.bass_utils import run_bass_kernel_spmd  # noqa: E402

F32 = mybir.dt.float32
F16 = mybir.dt.float16

N_CORES = 8
B, T, D = 4, 2048, 4096
A = 32              # algebra dim (hadamard size)
IN_O = 128
OUT_O = 128
TOK = (B * T) // N_CORES    # tokens per core = 1024
TC = 256                    # tokens per chunk
NCH = TOK // TC             # 4 chunks
CCOLS = TC * 32             # 8192 sbuf cols per chunk

_CACHE = {}


def _build_program():
    nc = bacc.Bacc(None, target_bir_lowering=False)

    x_d = nc.dram_tensor("x_shuf", [128, TOK * 32], F16, kind="ExternalInput")
    h1_d = nc.dram_tensor("h1m", [128, 128], F16, kind="ExternalInput")
    w2_d = nc.dram_tensor("w2m", [128, A * OUT_O], F16, kind="ExternalInput")
    y_d = nc.dram_tensor("y", [128, TOK * 32], F16, kind="ExternalOutput")

    with tile.TileContext(nc) as tc:
        with (
            tc.tile_pool(name="const", bufs=1) as constp,
            tc.tile_pool(name="xin", bufs=2) as xinp,
            tc.tile_pool(name="xm1", bufs=2) as xm1p,
            tc.tile_pool(name="xm2", bufs=2) as xm2p,
            tc.tile_pool(name="yp", bufs=2) as ypp,
            tc.tile_pool(name="ypt", bufs=2) as yptp,
            tc.tile_pool(name="ysb", bufs=2) as ysbp,
            tc.tile_pool(name="psA", bufs=2, space="PSUM") as psAp,
            tc.tile_pool(name="ps2", bufs=2, space="PSUM") as ps2p,
        ):
            h1_t = constp.tile([128, 128], F16)
            nc.sync.dma_start(out=h1_t[:], in_=h1_d[:])
            w2_t = constp.tile([128, A * OUT_O], F16)

            def load_x(c):
                subs = []
                for s in range(4):
                    xs_t = xinp.tile([128, 2048], F16, name="x_sub")
                    nc.sync.dma_start(
                        out=xs_t[:],
                        in_=x_d[:, c * CCOLS + s * 2048 : c * CCOLS + (s + 1) * 2048],
                    )
                    subs.append(xs_t)
                return subs

            def s1_round(ctx, r, pool=None):
                x_subs, xm1_32, xm2_32 = ctx
                ps = (pool.tile([128, 1024], F32, name="ps2t") if pool is not None
                      else psAp.tile([128, 1024], F32, name="psq"))
                for m in range(2):
                    off = (r % 2) * 1024 + m * 512
                    nc.tensor.matmul(
                        ps[:, m * 512 : m * 512 + 512],
                        h1_t[:],
                        x_subs[r // 2][:, off : off + 512],
                        start=True,
                        stop=True,
                    )
                nc.scalar.copy(
                    xm1_32[:, r * 512 : (r + 1) * 512].bitcast(F16), ps[:]
                )
                if r % 4 == 3:
                    s = r // 4
                    # xm2_32[i, w*64 + h*2 + s] = T(xm1 half s)
                    dstv = xm2_32.rearrange("p (w e q) -> p w q e", e=32, q=2)
                    nc.vector.transpose(
                        dstv[:, :, s : s + 1, :],
                        xm1_32[:, s * 2048 : (s + 1) * 2048],
                    )

            def s2_round(ctx, r):
                xm2_v, ypv = ctx
                ps2 = ps2p.tile([128, 1024], F32, name="ps2t")
                for j in range(4):
                    g = r * 4 + j
                    nc.tensor.matmul(
                        ps2[:, j * 256 : (j + 1) * 256],
                        w2_t[:, g * 128 : (g + 1) * 128],
                        xm2_v[:, :, 4 * g : 4 * g + 4],
                        start=True,
                        stop=True,
                    )
                # gather: yp[o, 32t + 4r + j] = ps2[o, 256j + t]
                geng = nc.vector.tensor_copy if r in (2, 5) else nc.scalar.copy
                geng(
                    ypv[:, :, 4 * r : 4 * r + 4],
                    ps2[:].rearrange("p (j t) -> p t j", j=4),
                )

            def s3_round(ctx, r, c, pool=None):
                ypt_t, y_sb = ctx
                ps = (pool.tile([128, 1024], F32, name="ps2t") if pool is not None
                      else psAp.tile([128, 1024], F32, name="psq"))
                for m in range(2):
                    off = r * 1024 + m * 512
                    nc.tensor.matmul(
                        ps[:, m * 512 : m * 512 + 512],
                        h1_t[:],
                        ypt_t[:, off : off + 512],
                        start=True,
                        stop=True,
                    )
                dst = y_sb[:, r * 1024 : (r + 1) * 1024]
                if r in (1, 4, 7):
                    nc.scalar.copy(dst, ps[:])
                else:
                    nc.vector.tensor_copy(dst, ps[:])
                if r % 2 == 1:
                    nc.sync.dma_start(
                        out=y_d[:, c * CCOLS + (r - 1) * 1024 : c * CCOLS + (r + 1) * 1024],
                        in_=y_sb[:, (r - 1) * 1024 : (r + 1) * 1024],
                    )

            # software-pipelined, round-interleaved: S1(c) | S2(c-1) | S3(c-2)
            xs = {0: load_x(0)}
            nc.sync.dma_start(out=w2_t[:], in_=w2_d[:])
            c1 = {}   # chunk -> stage1 ctx (x, xm1_32, xm2_32)
            c2 = {}   # chunk -> (xm2_t, yp_t, ypt_t)
            c3 = {}   # chunk -> (ypt_t, y_sb)
            for it in range(NCH + 2):
                if it < NCH:
                    xm1_t = xm1p.tile([128, CCOLS], F16, name="xm1_t")
                    xm2_t = xm2p.tile([128, CCOLS], F16, name="xm2_t")
                    c1[it] = (xs.pop(it), xm1_t[:].bitcast(F32), xm2_t[:].bitcast(F32))
                    c2[it] = [xm2_t, None, None]
                if 0 <= it - 1 < NCH:
                    yp_t = ypp.tile([128, CCOLS], F16, name="yp_t")
                    ypt_t = yptp.tile([128, CCOLS], F16, name="ypt_t")
                    c2[it - 1][1] = yp_t
                    c2[it - 1][2] = ypt_t
                if 0 <= it - 2 < NCH:
                    y_sb = ysbp.tile([128, CCOLS], F16, name="y_sb")
                    c3[it - 2] = (c2[it - 2][2], y_sb)

                s2ctx = None
                if 0 <= it - 1 < NCH:
                    xm2_t, yp_t, _ = c2[it - 1]
                    s2ctx = (
                        xm2_t[:].rearrange("p (w q) -> p w q", q=128),
                        yp_t[:].rearrange("p (t h) -> p t h", h=32),
                    )
                s2_active = s2ctx is not None
                for r in range(8):
                    if it < NCH:
                        p1 = None if s2_active else (ps2p if r % 2 else None)
                        s1_round(c1[it], r, p1)
                    if s2_active:
                        s2_round(s2ctx, r)
                    if 0 <= it - 2 < NCH:
                        p3 = None if s2_active else (ps2p if r % 2 == 0 else None)
                        s3_round(c3[it - 2], r, it - 2, p3)
                    if r == 3 and it + 1 < NCH:
                        xs[it + 1] = load_x(it + 1)

                # T2 for chunk it-1 (needs all gathers of that chunk)
                if 0 <= it - 1 < NCH:
                    _, yp_t, ypt_t = c2[it - 1]
                    for s in range(2):
                        nc.vector.transpose(
                            ypt_t[:, s * 4096 : (s + 1) * 4096],
                            yp_t[:, s * 4096 : (s + 1) * 4096],
                        )
                if 0 <= it - 2 < NCH:
                    c3.pop(it - 2)
                    c2.pop(it - 2)

    nc.compile()
    return nc


def _host_prep(x, weight_packed, beta, H):
    x = np.asarray(x, dtype=np.float32)
    weight_packed = np.asarray(weight_packed, dtype=np.uint8)
    beta = np.asarray(beta, dtype=np.float32)
    H = np.asarray(H, dtype=np.float32)

    hpm = np.where(H > 0, 1.0, -1.0).astype(np.float32)

    # unpack ternary weights exactly like the reference
    p = weight_packed
    v0 = ((p >> 6) & 3).astype(np.int8) - 1
    v1 = ((p >> 4) & 3).astype(np.int8) - 1
    v2 = ((p >> 2) & 3).astype(np.int8) - 1
    v3 = (p & 3).astype(np.int8) - 1
    w = np.stack([v0, v1, v2, v3], axis=-1).reshape(A, OUT_O, IN_O).astype(np.float32)

    # stage 1/3 stationary: h1m[32d+g, 32d'+h] = delta_dd' * hpm[g,h]
    h1m = np.zeros((4, A, 4, A), dtype=np.float32)
    for d in range(4):
        h1m[d, :, d, :] = hpm
    h1m = h1m.reshape(128, 128).astype(np.float16)

    # stage 2 stationary: w2m[i, 128g + o] = w[g,o,i] * beta[o] / 32
    w2 = w * (beta[None, :, None] / 32.0)
    w2m = np.ascontiguousarray(w2.transpose(2, 0, 1).reshape(IN_O, A * OUT_O)).astype(
        np.float16
    )

    # per-core pre-shuffled x (fp16):
    # x_shuf[32d+g, 8192c + 64u + 2*i_hi + v] = x[t0 + 256c + 2u + v, 128g+32d+i_hi]
    xf = x.reshape(B * T, D).astype(np.float16)
    x_shards = []
    for core in range(N_CORES):
        xc = xf[core * TOK : (core + 1) * TOK]          # [TOK, 4096]
        # t = 256c + 4w + 2*half + va ; col = 8192c + 4096*half + 64w + 2*i_hi + va
        xc = xc.reshape(NCH, TC // 4, 2, 2, A, 4, 32)   # [c, w, half, va, g, d, i_hi]
        xc = xc.transpose(5, 4, 0, 2, 1, 6, 3)          # [d, g, c, half, w, i_hi, va]
        x_shards.append(np.ascontiguousarray(xc.reshape(128, TOK * 32)))

    return x_shards, h1m, w2m


def kernel(x, weight_packed, beta, H):
    x_shards, h1m, w2m = _host_prep(x, weight_packed, beta, H)

    if "nc" not in _CACHE:
        _CACHE["nc"] = _build_program()
    nc = _CACHE["nc"]

    in_maps = [
        {"x_shuf": x_shards[c], "h1m": h1m, "w2m": w2m} for c in range(N_CORES)
    ]
    res = run_bass_kernel_spmd(nc, in_maps, core_ids=list(range(N_CORES)))

    # y_d[32d3'+h', 8192c + 32t + o_hi] = y[t0+256c+t, 128h' + 32d3' + o_hi]
    out = np.empty((B * T, D), dtype=np.float32)
    for core in range(N_CORES):
        yd = np.asarray(res.results[core]["y"])          # [128, TOK*32] fp16
        arr = yd.reshape(4, A, NCH, TC, 32)              # [d3', h', c, t, o_hi]
        arr = arr.transpose(2, 3, 1, 0, 4).reshape(TOK, D)  # [c,t][h',d3',o_hi]
        out[core * TOK : (core + 1) * TOK] = arr.astype(np.float32)
    return out.reshape(B, T, D)
